# revision 1
# baseline (speedup 1.0000x reference)
"""CARAFE++ content-aware upsampling kernel for Trainium2 (8 NeuronCores), v3.

Per-core pipeline (4 batches x 2 row-halves):
  1. conv1 as matmul (fp16) + relu -> featd: W-padded feat in partitions 0-63,
     one-row-shifted copy in partitions 64-127 (for conv2 row-pair packing)
  2. conv2 as 6 shifted matmuls per 8-row tile (3x K=128 row-pairs + 3x K=64),
     + bias + exp -> wk
  3. softmax denominators via block-ones matmul; reciprocal; broadcast back to
     100 rows via a 0/1-matrix matmul; wk_n = wk * recip (all per conv tile)
  4. XBAR dma-transpose wk_n -> wkT (pixel-major), 1 per conv tile
  5. per block-pair: two gpsimd local_scatters build band-matrix-transpose
     layouts; ONE XBAR dma-transpose yields 24 S panels; 12 accumulated
     fp16 matmuls reassemble; evict fp16 + store
All XBAR transposes serialize on the sync HWDGE ring (HW hazard otherwise);
loads/stores ride the scalar ring. XBAR writes/reads get manual deps (Tile
does not track InstDmaTransposeAnt).
"""
import sys

sys.path.insert(0, "/opt/trn_rl_repo")

import numpy as np
from contextlib import ExitStack

import concourse.bass as bass
import concourse.bacc as bacc
import concourse.tile as tile
from concourse.tile import add_dep_helper
from concourse import mybir
from concourse.bass_utils import run_bass_kernel_spmd

B, C, H, W = 4, 256, 64, 64
SCALE, K, COMP, G = 2, 5, 4, 1
MID = 64
ENC = 100          # K*K*SCALE*SCALE
NROW = 36          # x rows per core (32 + 2 halo each side)
NPX = NROW * W     # 2304
FROW = 34          # feat rows r0-1 .. r0+32
FPW = W + 2        # 66, feat row W-padded
NBLK = 16          # output row-pair blocks per core
NJB = 18           # x row-pair panels per core

f32 = mybir.dt.float32
f16 = mybir.dt.float16
i16 = mybir.dt.int16

# const_f16 blob column layout
C_WC0, C_WC1, C_WEP, C_WES, C_ONES, C_E4 = 0, 64, 128, 428, 728, 732
C_TOT = 832

_CACHE = {}


def _build_idxs():
    """Per-partition scatter indices. Partition = out-center px (rt, w); slot
    = (p, dy, dx) wk channel order; dest = (dj*4+p)*128 + rb*64 + wi so the
    XBAR panel-major transpose yields S panels grouped (dj, p)."""
    idxs = np.full((128, 100), -1, np.int16)
    for rt in range(2):
        for w in range(W):
            part = rt * W + w
            for p in range(4):
                for dy in range(-2, 3):
                    dj = (rt + dy + 2) // 2
                    rb = (rt + dy) % 2
                    for dx in range(-2, 3):
                        wi = w + dx
                        if 0 <= wi < W:
                            slot = p * 25 + (dy + 2) * 5 + (dx + 2)
                            idxs[part, slot] = (dj * 4 + p) * 128 + rb * 64 + wi
    return idxs


def _build_nc():
    nc = bacc.Bacc("TRN2", target_bir_lowering=False, debug=False, num_devices=8)

    # ---- DRAM I/O (per-core shapes)
    d_x = nc.dram_tensor("x", [128, 2 * NPX], f16, kind="ExternalInput")
    d_xt = nc.dram_tensor("xt", [128, NJB * 2 * 128], f16, kind="ExternalInput")
    d_cst = nc.dram_tensor("cst", [128, C_TOT], f16, kind="ExternalInput")
    d_cstf = nc.dram_tensor("cstf", [128, 2], f32, kind="ExternalInput")
    d_idx = nc.dram_tensor("idx", [128, 100], i16, kind="ExternalInput")
    d_out = nc.dram_tensor("out", [C, 32 * 256], f16, kind="ExternalOutput")

    with tile.TileContext(nc) as tc, ExitStack() as ctx:
        sb1 = ctx.enter_context(tc.tile_pool(name="sb1", bufs=1))
        sbw = ctx.enter_context(tc.tile_pool(name="sbw", bufs=2))
        ps = ctx.enter_context(tc.tile_pool(name="ps", bufs=1, space="PSUM"))

        # ---- loads (scalar ring; xt on sync ring before any transposes)
        cst = sb1.tile([128, C_TOT], f16, tag="cst")
        cstf = sb1.tile([128, 2], f32, tag="cstf")
        sidx = sb1.tile([128, 100], i16, tag="sidx")
        xall = sb1.tile([128, 2 * NPX], f16, tag="xall")
        xt = sb1.tile([128, NJB, 2, 128], f16, tag="xt")
        nc.scalar.dma_start(out=cst, in_=d_cst[:])
        nc.scalar.dma_start(out=cstf, in_=d_cstf[:])
        nc.scalar.dma_start(out=sidx, in_=d_idx[:])
        nc.scalar.dma_start(out=xall, in_=d_x[:])
        nc.sync.dma_start(
            out=xt, in_=d_xt[:].rearrange("q (j c m) -> q j c m", j=NJB, c=2))

        bc = cstf[0:MID, 0:1]
        be = cstf[0:ENC, 1:2]
        wc0 = cst[:, C_WC0:C_WC0 + 64]
        wc1 = cst[:, C_WC1:C_WC1 + 64]
        ones = cst[0:ENC, C_ONES:C_ONES + 4]
        e4 = cst[0:4, C_E4:C_E4 + ENC]

        # warm the Exp activation table during load shadow
        scratch = sb1.tile([ENC, 1], f32, tag="scratch")
        nc.scalar.activation(out=scratch[:], in_=be,
                             func=mybir.ActivationFunctionType.Exp,
                             bias=be, scale=1.0)

        featd = sb1.tile([128, FROW * FPW], f16, tag="featd")
        nc.vector.memset(featd, 0.0)
        wk = sb1.tile([ENC, 2048], f16, tag="wk")
        wkn = sb1.tile([112, 2048], f16, tag="wkn")
        nc.vector.memset(wkn[96:112, :], 0.0)
        wkT = sb1.tile([128, NBLK, 112], f16, tag="wkT")
        wkT_x = []

        # ---- conv1 tile: 1x1 conv (256->64) + relu -> featd (both halves)
        def conv1_tile(nt):
            n0 = W + nt * 512
            n = min(512, 2240 - n0)
            pf = ps.tile([MID, 512], f32, tag="pf", bufs=2, name="pf")
            nc.tensor.matmul(pf[:, :n], wc0, xall[:, n0:n0 + n],
                             start=True, stop=False)
            nc.tensor.matmul(pf[:, :n], wc1, xall[:, NPX + n0:NPX + n0 + n],
                             start=False, stop=True)
            fp0 = n0 // W - 1
            nr = n // W
            src = pf[:, :n].rearrange("m (r w) -> m r w", w=W)
            fd1h = featd[0:64]
            dst1 = bass.AP(
                tensor=fd1h.tensor, offset=fd1h.offset + fp0 * FPW + 1,
                ap=[fd1h.ap[0], [FPW, nr], [1, W]],
            )
            nc.scalar.activation(out=dst1, in_=src,
                                 func=mybir.ActivationFunctionType.Relu,
                                 bias=bc, scale=1.0)
            fd2 = featd[64:128]
            if fp0 == 0:
                src2 = bass.AP(tensor=pf.tensor, offset=pf.offset + W,
                               ap=[pf.ap[0], [W, nr - 1], [1, W]])
                dst2 = bass.AP(tensor=fd2.tensor, offset=fd2.offset + 1,
                               ap=[fd2.ap[0], [FPW, nr - 1], [1, W]])
            else:
                src2 = bass.AP(tensor=pf.tensor, offset=pf.offset,
                               ap=[pf.ap[0], [W, nr], [1, W]])
                dst2 = bass.AP(tensor=fd2.tensor,
                               offset=fd2.offset + (fp0 - 1) * FPW + 1,
                               ap=[fd2.ap[0], [FPW, nr], [1, W]])
            nc.scalar.activation(out=dst2, in_=src2,
                                 func=mybir.ActivationFunctionType.Relu,
                                 bias=bc, scale=1.0)

        # ---- conv2 tile: 3x3 conv + exp; sums; recip; wk_n; XBAR wkT
        def conv2_tile(nt):
            h0 = nt * 8
            sl = slice(nt * 512, (nt + 1) * 512)
            pw = ps.tile([ENC, 512], f32, tag="pw", bufs=1, name="pw")
            for j in range(3):
                rhs = bass.AP(
                    tensor=featd.tensor, offset=featd.offset + h0 * FPW + j,
                    ap=[featd.ap[0], [FPW, 8], [1, W]],
                )
                nc.tensor.matmul(pw[:], cst[:, C_WEP + j * ENC:C_WEP + (j + 1) * ENC],
                                 rhs, start=(j == 0), stop=False)
            fd1 = featd[0:64]
            for j in range(3):
                rhs = bass.AP(
                    tensor=fd1.tensor,
                    offset=fd1.offset + (h0 + 2) * FPW + j,
                    ap=[fd1.ap[0], [FPW, 8], [1, W]],
                )
                nc.tensor.matmul(pw[:], cst[0:64, C_WES + j * ENC:C_WES + (j + 1) * ENC],
                                 rhs, start=False, stop=(j == 2))
            nc.scalar.activation(out=wk[:, sl], in_=pw[:],
                                 func=mybir.ActivationFunctionType.Exp,
                                 bias=be, scale=1.0)
            paux = ps.tile([ENC, 512], f32, tag="paux", bufs=1, name="paux")
            nc.tensor.matmul(paux[0:4, :], ones, wk[:, sl], start=True, stop=True)
            recip4 = sbw.tile([4, 512], f16, tag="recip4", bufs=2, name="recip4")
            with nc.allow_low_precision(reason="softmax recip fine in fp16"):
                nc.vector.reciprocal(recip4[:], paux[0:4, :])
            paux2 = ps.tile([ENC, 512], f32, tag="paux", bufs=1, name="paux2")
            nc.tensor.matmul(paux2[:], e4, recip4[:], start=True, stop=True)
            mul_i = nc.vector.tensor_mul(wkn[0:ENC, sl], wk[:, sl], paux2[:])
            wx = nc.sync.dma_start_transpose(
                out=wkT[:, nt * 4:(nt + 1) * 4, :], in_=wkn[:, sl])
            # Tile does not dep-track XBAR transposes: manual edge
            add_dep_helper(wx.ins, mul_i.ins, reason="xbar wkT reads wkn")
            wkT_x.append(wx)

        conv1_tile(0)
        conv1_tile(1)
        for nt in range(4):
            if nt + 2 < 5:
                conv1_tile(nt + 2)
            conv2_tile(nt)

        # ---- block pairs: scatter x2, one XBAR transpose, 12 matmuls, evict
        sdst_reader = [None, None]
        t2list = [None] * 8
        osegs = [None, None]

        def scatter(t):
            g = t // 2
            if t % 2 == 0:
                sd = sbw.tile([128, 2, 1536], f16, tag="sdst", bufs=2, name="sd")
                t2list[g] = sd
            sd = t2list[g]
            sc = nc.gpsimd.local_scatter(
                out_ap=sd[:, t % 2, :], data_ap=wkT[:, t, 0:100], idxs_ap=sidx[:],
                channels=128, num_elems=1536, num_idxs=100,
            )
            add_dep_helper(sc.ins, wkT_x[t // 4].ins,
                           reason="scatter reads xbar wkT")
            if sdst_reader[g % 2] is not None:
                add_dep_helper(sc.ins, sdst_reader[g % 2].ins,
                               reason="WAR: scatter pair overwrites xbar-read sdst")
            return sc

        def transpose_pair(g, sc0, sc1):
            T2 = sbw.tile([128, 24, 128], f16, tag="T2", bufs=3, name="T2")
            tx = nc.sync.dma_start_transpose(out=T2[:], in_=t2list[g][:])
            add_dep_helper(tx.ins, sc0.ins, reason="xbar reads scatter0")
            add_dep_helper(tx.ins, sc1.ins, reason="xbar reads scatter1")
            sdst_reader[g % 2] = tx
            return T2, tx

        def reassemble(t, T2, tx):
            for ch in range(2):
                po = ps.tile([128, 512], f32, tag="po", bufs=4, name="po")
                for dj in range(3):
                    pan = (t % 2) * 12 + dj * 4
                    mm_i = nc.tensor.matmul(
                        po[:], xt[:, t + dj, ch, :], T2[:, pan:pan + 4, :],
                        start=(dj == 0), stop=(dj == 2),
                    )
                    if ch == 0 and dj == 0:
                        add_dep_helper(mm_i.ins, tx.ins, reason="PE reads xbar T")
                if t % 2 == 0 and ch == 0:
                    osegs[0] = sbw.tile([128, 1024], f16, tag="oseg0", bufs=2,
                                        name="oseg0")
                    osegs[1] = sbw.tile([128, 1024], f16, tag="oseg1", bufs=2,
                                        name="oseg1")
                dst = bass.AP(
                    tensor=osegs[ch].tensor,
                    offset=osegs[ch].offset + (t % 2) * 512,
                    ap=[osegs[ch].ap[0], [256, 2], [4, 64], [1, 4]],
                )
                srcp = bass.AP(tensor=po.tensor, offset=po.offset,
                               ap=[po.ap[0], [64, 2], [1, 64], [128, 4]])
                if ch == 0:
                    nc.scalar.activation(out=dst, in_=srcp,
                                         func=mybir.ActivationFunctionType.Copy,
                                         scale=1.0)
                else:
                    nc.vector.tensor_copy(dst, srcp)
            last = mm_i
            if t % 2 == 1:
                for ch in range(2):
                    nc.scalar.dma_start(
                        out=d_out[ch * 128:(ch + 1) * 128,
                                  (t - 1) * 512:(t + 1) * 512],
                        in_=osegs[ch][:],
                    )
            return last

        # software-pipelined emission: pair g+1's scatters+transpose are
        # emitted before pair g's matmuls
        pend = {}
        last_pe = [None, None, None]   # T2 slot -> last PE reader
        sc0 = scatter(0)
        sc1 = scatter(1)
        pend[0] = transpose_pair(0, sc0, sc1)
        for g in range(8):
            if g + 1 < 8:
                sc0 = scatter(2 * g + 2)
                sc1 = scatter(2 * g + 3)
                pend[g + 1] = transpose_pair(g + 1, sc0, sc1)
                if last_pe[(g + 1) % 3] is not None:
                    add_dep_helper(pend[g + 1][1].ins, last_pe[(g + 1) % 3].ins,
                                   reason="WAR: xbar overwrites PE-read T2")
            T2, tx = pend[g]
            reassemble(2 * g, T2, tx)
            last_pe[g % 3] = reassemble(2 * g + 1, T2, tx)

    nc.compile()
    return nc


def _host_prep(x, W_comp, b_comp, W_enc, b_enc):
    """Build per-core input maps (layout/dtype prep only)."""
    idxs = _build_idxs()
    cst = np.zeros((128, C_TOT), np.float16)
    cst[0:128, C_WC0:C_WC0 + 64] = W_comp.T[0:128]
    cst[0:128, C_WC1:C_WC1 + 64] = W_comp.T[128:256]
    for j in range(3):
        cst[0:64, C_WEP + j * ENC:C_WEP + (j + 1) * ENC] = W_enc[:, :, 0, j].T
        cst[64:128, C_WEP + j * ENC:C_WEP + (j + 1) * ENC] = W_enc[:, :, 1, j].T
        cst[0:64, C_WES + j * ENC:C_WES + (j + 1) * ENC] = W_enc[:, :, 2, j].T
    for p in range(4):
        cst[C_ONES * 0 + p * 25:(p + 1) * 25, C_ONES + p] = 1.0
        cst[p, C_E4 + p * 25:C_E4 + (p + 1) * 25] = 1.0
    cstf = np.zeros((128, 2), np.float32)
    cstf[0:MID, 0] = b_comp
    cstf[0:ENC, 1] = b_enc

    xp = np.pad(x, ((0, 0), (0, 0), (2, 2), (0, 0)))   # (B, C, 68, 64)
    in_maps = []
    for core in range(8):
        b, half = core // 2, core % 2
        r0 = 32 * half
        xs = np.ascontiguousarray(
            xp[b, :, r0:r0 + NROW, :].reshape(C, NPX)).astype(np.float16)
        xflat = np.ascontiguousarray(xs.reshape(2, 128, NPX).transpose(1, 0, 2)
                                     ).reshape(128, 2 * NPX)
        xtc = np.ascontiguousarray(
            xs.reshape(2, 128, NJB, 128).transpose(3, 2, 0, 1)
        ).reshape(128, NJB * 2 * 128)
        in_maps.append(dict(x=xflat, xt=xtc, cst=cst, cstf=cstf, idx=idxs))
    return in_maps


def kernel(x, W_comp, b_comp, W_enc, b_enc):
    x = np.asarray(x, np.float32)
    W_comp = np.asarray(W_comp, np.float32)
    b_comp = np.asarray(b_comp, np.float32)
    W_enc = np.asarray(W_enc, np.float32)
    b_enc = np.asarray(b_enc, np.float32)

    if "nc" not in _CACHE:
        _CACHE["nc"] = _build_nc()
    nc = _CACHE["nc"]

    in_maps = _host_prep(x, W_comp, b_comp, W_enc, b_enc)
    res = run_bass_kernel_spmd(nc, in_maps, core_ids=list(range(8)))

    out = np.empty((B, C, 128, 128), np.float32)
    for core in range(8):
        b, half = core // 2, core % 2
        seg = res.results[core]["out"].astype(np.float32)   # (256, 8192) f16
        out[b, :, 64 * half:64 * (half + 1), :] = seg.reshape(C, 64, 128)
    return out


if __name__ == "__main__":
    d = np.load("/tmp/carafe_ref.npz")
    expected = d["expected"]
    out = kernel(**{k: d[k] for k in ["x", "W_comp", "b_comp", "W_enc", "b_enc"]})
    err = np.abs(out - expected)
    scale = np.abs(expected).max()
    print(f"absmax err: {err.max():.4e}  rel: {err.max()/scale:.4e}")



# revision 5
# speedup vs baseline: 1.1382x; 1.1382x over previous
"""CARAFE++ content-aware upsampling kernel for Trainium2 (8 NeuronCores), v4.

Per-core pipeline (4 batches x 2 row-halves):
  1. conv1 as matmul (fp16) + relu -> featd: W-padded feat in partitions 0-63,
     one-row-shifted copy in partitions 64-127 (for conv2 row-pair packing)
  2. conv2 as 6 shifted matmuls per 8-row tile (3x K=128 row-pairs + 3x K=64),
     + bias + exp -> wk
  3. softmax denominators via block-ones matmul; reciprocal; broadcast back to
     100 rows via a 0/1-matrix matmul; wk_n = wk * recip (all per conv tile)
  4. XBAR dma-transpose wk_n -> wkT (pixel-major), 1 per conv tile
  5. per block-pair: two gpsimd local_scatters build band-matrix-transpose
     layouts; ONE XBAR dma-transpose yields 24 S panels; 12 accumulated
     fp16 matmuls reassemble; evict fp16 contiguous (no interleave; the host
     does the pixel-shuffle) + grouped stores
v4 changes vs v3: x load split in 4 chunks (early conv1 start), softmax chain
software-pipelined so the DVE reciprocal never blocks the tensor queue,
tile-3 chain tail deferred past the first reassembly pair, wkT transposes
interleaved into the T2 stream just-in-time, contiguous evictions with
grouped (4-block) stores, host-side output unshuffle.
All XBAR transposes serialize on the sync HWDGE ring (HW hazard otherwise);
loads/stores ride the scalar ring. XBAR writes/reads get manual deps (Tile
does not track InstDmaTransposeAnt).
"""
import sys

sys.path.insert(0, "/opt/trn_rl_repo")

import numpy as np
from contextlib import ExitStack

import concourse.bass as bass
import concourse.bacc as bacc
import concourse.tile as tile
from concourse.tile import add_dep_helper
from concourse import mybir
from concourse.bass_utils import run_bass_kernel_spmd

B, C, H, W = 4, 256, 64, 64
SCALE, K, COMP, G = 2, 5, 4, 1
MID = 64
ENC = 100          # K*K*SCALE*SCALE
NROW = 36          # x rows per core (32 + 2 halo each side)
NPX = NROW * W     # 2304
FROW = 34          # feat rows r0-1 .. r0+32
FPW = W + 2        # 66, feat row W-padded
NBLK = 16          # output row-pair blocks per core
NJB = 18           # x row-pair panels per core
XSPLIT = 1600      # x column split point per half (covers conv1 tiles 0-2)

f32 = mybir.dt.float32
f16 = mybir.dt.float16
i16 = mybir.dt.int16

# const_f16 blob column layout
C_WC0, C_WC1, C_WEP, C_WES, C_ONES, C_E4 = 0, 64, 128, 428, 728, 732
C_TOT = 832

_CACHE = {}


def _build_idxs():
    """Per-partition scatter indices. Partition = out-center px (rt, w); slot
    = (p, dy, dx) wk channel order; dest = (dj*4+p)*128 + rb*64 + wi so the
    XBAR panel-major transpose yields S panels grouped (dj, p)."""
    idxs = np.full((128, 100), -1, np.int16)
    for rt in range(2):
        for w in range(W):
            part = rt * W + w
            for p in range(4):
                for dy in range(-2, 3):
                    dj = (rt + dy + 2) // 2
                    rb = (rt + dy) % 2
                    for dx in range(-2, 3):
                        wi = w + dx
                        if 0 <= wi < W:
                            slot = p * 25 + (dy + 2) * 5 + (dx + 2)
                            idxs[part, slot] = (dj * 4 + p) * 128 + rb * 64 + wi
    return idxs


def _build_nc():
    nc = bacc.Bacc("TRN2", target_bir_lowering=False, debug=False, num_devices=8)

    # ---- DRAM I/O (per-core shapes)
    d_x = nc.dram_tensor("x", [128, 2 * NPX], f16, kind="ExternalInput")
    d_xt = nc.dram_tensor("xt", [128, NJB * 2 * 128], f16, kind="ExternalInput")
    d_cst = nc.dram_tensor("cst", [128, C_TOT], f16, kind="ExternalInput")
    d_cstf = nc.dram_tensor("cstf", [128, 2], f32, kind="ExternalInput")
    d_idx = nc.dram_tensor("idx", [128, 100], i16, kind="ExternalInput")
    d_out = nc.dram_tensor("out", [128, 2 * NBLK * 512], f16,
                           kind="ExternalOutput")

    with tile.TileContext(nc) as tc, ExitStack() as ctx:
        sb1 = ctx.enter_context(tc.tile_pool(name="sb1", bufs=1))
        sbw = ctx.enter_context(tc.tile_pool(name="sbw", bufs=2))
        ps = ctx.enter_context(tc.tile_pool(name="ps", bufs=1, space="PSUM"))

        # ---- loads. scalar ring: weights first, then x halves (conv1 tile 0
        # only gates on the "a" chunks); sync ring: xt (consumed late).
        cst = sb1.tile([128, C_TOT], f16, tag="cst")
        cstf = sb1.tile([128, 2], f32, tag="cstf")
        sidx = sb1.tile([128, 100], i16, tag="sidx")
        xh = []   # [h][chunk] tiles
        nc.scalar.dma_start(out=cst, in_=d_cst[:])
        for h in range(2):
            a = sb1.tile([128, XSPLIT], f16, tag=f"xh{h}a")
            b = sb1.tile([128, NPX - XSPLIT], f16, tag=f"xh{h}b")
            xh.append((a, b))
        for h in range(2):
            nc.scalar.dma_start(out=xh[h][0],
                                in_=d_x[:, h * NPX:h * NPX + XSPLIT])
        nc.scalar.dma_start(out=cstf, in_=d_cstf[:])
        nc.scalar.dma_start(out=sidx, in_=d_idx[:])
        for h in range(2):
            nc.scalar.dma_start(out=xh[h][1],
                                in_=d_x[:, h * NPX + XSPLIT:(h + 1) * NPX])
        xt = sb1.tile([128, NJB, 2, 128], f16, tag="xt")
        nc.sync.dma_start(
            out=xt, in_=d_xt[:].rearrange("q (j c m) -> q j c m", j=NJB, c=2))

        bc = cstf[0:MID, 0:1]
        be = cstf[0:ENC, 1:2]
        wc0 = cst[:, C_WC0:C_WC0 + 64]
        wc1 = cst[:, C_WC1:C_WC1 + 64]
        ones = cst[0:ENC, C_ONES:C_ONES + 4]
        e4 = cst[0:4, C_E4:C_E4 + ENC]

        # warm the Exp activation table during load shadow
        scratch = sb1.tile([ENC, 1], f32, tag="scratch")
        nc.scalar.activation(out=scratch[:], in_=be,
                             func=mybir.ActivationFunctionType.Exp,
                             bias=be, scale=1.0)

        featd = sb1.tile([128, FROW * FPW], f16, tag="featd")
        nc.vector.memset(featd, 0.0)
        wk = sb1.tile([ENC, 2048], f16, tag="wk")
        wkn = sb1.tile([112, 2048], f16, tag="wkn")
        nc.vector.memset(wkn[96:112, :], 0.0)
        wkTs = [None] * 4  # per conv tile [128, 4, 112]
        wkT_x = [None] * 4
        mul_is = [None] * 4

        def xsrc(h, n0, n):
            """x slice [n0, n0+n) of half h from the right chunk tile."""
            if n0 + n <= XSPLIT:
                return xh[h][0][:, n0:n0 + n]
            return xh[h][1][:, n0 - XSPLIT:n0 - XSPLIT + n]

        # ---- conv1 tile: 1x1 conv (256->64) + relu -> featd (both halves)
        def conv1_tile(nt):
            n0 = W + nt * 512
            n = min(512, 2240 - n0)
            pf = ps.tile([MID, 512], f32, tag="pf", bufs=2, name="pf")
            nc.tensor.matmul(pf[:, :n], wc0, xsrc(0, n0, n),
                             start=True, stop=False)
            nc.tensor.matmul(pf[:, :n], wc1, xsrc(1, n0, n),
                             start=False, stop=True)
            fp0 = n0 // W - 1
            nr = n // W
            src = pf[:, :n].rearrange("m (r w) -> m r w", w=W)
            fd1h = featd[0:64]
            dst1 = bass.AP(
                tensor=fd1h.tensor, offset=fd1h.offset + fp0 * FPW + 1,
                ap=[fd1h.ap[0], [FPW, nr], [1, W]],
            )
            nc.scalar.activation(out=dst1, in_=src,
                                 func=mybir.ActivationFunctionType.Relu,
                                 bias=bc, scale=1.0)
            fd2 = featd[64:128]
            if fp0 == 0:
                src2 = bass.AP(tensor=pf.tensor, offset=pf.offset + W,
                               ap=[pf.ap[0], [W, nr - 1], [1, W]])
                dst2 = bass.AP(tensor=fd2.tensor, offset=fd2.offset + 1,
                               ap=[fd2.ap[0], [FPW, nr - 1], [1, W]])
            else:
                src2 = bass.AP(tensor=pf.tensor, offset=pf.offset,
                               ap=[pf.ap[0], [W, nr], [1, W]])
                dst2 = bass.AP(tensor=fd2.tensor,
                               offset=fd2.offset + (fp0 - 1) * FPW + 1,
                               ap=[fd2.ap[0], [FPW, nr], [1, W]])
            nc.scalar.activation(out=dst2, in_=src2,
                                 func=mybir.ActivationFunctionType.Relu,
                                 bias=bc, scale=1.0)

        # ---- conv2 pieces, split so the softmax chain pipelines across
        # tiles without ever stalling the tensor queue.
        def conv2_mm(nt):
            h0 = nt * 8
            sl = slice(nt * 512, (nt + 1) * 512)
            pw = ps.tile([ENC, 512], f32, tag="pw", bufs=1, name="pw")
            for j in range(3):
                rhs = bass.AP(
                    tensor=featd.tensor, offset=featd.offset + h0 * FPW + j,
                    ap=[featd.ap[0], [FPW, 8], [1, W]],
                )
                nc.tensor.matmul(pw[:], cst[:, C_WEP + j * ENC:C_WEP + (j + 1) * ENC],
                                 rhs, start=(j == 0), stop=False)
            fd1 = featd[0:64]
            for j in range(3):
                rhs = bass.AP(
                    tensor=fd1.tensor,
                    offset=fd1.offset + (h0 + 2) * FPW + j,
                    ap=[fd1.ap[0], [FPW, 8], [1, W]],
                )
                nc.tensor.matmul(pw[:], cst[0:64, C_WES + j * ENC:C_WES + (j + 1) * ENC],
                                 rhs, start=False, stop=(j == 2))
            return pw

        def conv2_exp(nt, pw):
            sl = slice(nt * 512, (nt + 1) * 512)
            nc.scalar.activation(out=wk[:, sl], in_=pw[:],
                                 func=mybir.ActivationFunctionType.Exp,
                                 bias=be, scale=1.0)

        def conv2_ones(nt):
            sl = slice(nt * 512, (nt + 1) * 512)
            paux = ps.tile([ENC, 512], f32, tag="paux", bufs=1, name="paux")
            nc.tensor.matmul(paux[0:4, :], ones, wk[:, sl], start=True, stop=True)
            recip4 = sbw.tile([4, 512], f16, tag="recip4", bufs=2, name="recip4")
            with nc.allow_low_precision(reason="softmax recip fine in fp16"):
                nc.vector.reciprocal(recip4[:], paux[0:4, :])
            return recip4

        def conv2_tail(nt, recip4):
            """e4 broadcast matmul + normalize + XBAR wkT for tile nt."""
            sl = slice(nt * 512, (nt + 1) * 512)
            paux2 = ps.tile([ENC, 512], f32, tag="paux", bufs=1, name="paux2")
            nc.tensor.matmul(paux2[:], e4, recip4[:], start=True, stop=True)
            mul_i = nc.vector.tensor_mul(wkn[0:ENC, sl], wk[:, sl], paux2[:])
            mul_is[nt] = mul_i

        def wkt_x(nt):
            sl = slice(nt * 512, (nt + 1) * 512)
            wkT = sbw.tile([128, 4, 112], f16, tag="wkT", bufs=4, name="wkT")
            wkTs[nt] = wkT
            wx = nc.sync.dma_start_transpose(out=wkT[:], in_=wkn[:, sl])
            # Tile does not dep-track XBAR transposes: manual edge
            add_dep_helper(wx.ins, mul_is[nt].ins, reason="xbar wkT reads wkn")
            wkT_x[nt] = wx

        # ---- conv phase emission: tensor queue stays dense; each tile's
        # reciprocal latency is hidden under the next tile's conv matmuls.
        conv1_tile(0)
        conv1_tile(1)
        recips = [None] * 4
        pw0 = conv2_mm(0)
        conv2_exp(0, pw0)
        conv1_tile(2)
        recips[0] = conv2_ones(0)
        pw1 = conv2_mm(1)
        conv2_tail(0, recips[0])
        wkt_x(0)
        conv2_exp(1, pw1)
        conv1_tile(3)
        recips[1] = conv2_ones(1)
        pw2 = conv2_mm(2)
        conv2_tail(1, recips[1])
        wkt_x(1)
        conv2_exp(2, pw2)
        conv1_tile(4)
        recips[2] = conv2_ones(2)
        pw3 = conv2_mm(3)
        conv2_tail(2, recips[2])
        conv2_exp(3, pw3)
        recips[3] = conv2_ones(3)
        # tile 3's tail (e4 matmul gated on a 3.3us DVE reciprocal) is
        # deferred into the reassembly loop so it never stalls the PE.

        # ---- reassembly: per block scatter; per pair one XBAR transpose,
        # 12 matmuls, contiguous fp16 evictions, grouped stores.
        sdst_reader = [None, None, None]
        t2list = [None] * 8
        osegs = [None] * 4

        def scatter(t):
            g = t // 2
            if t % 2 == 0:
                sd = sbw.tile([128, 2, 1536], f16, tag="sdst", bufs=3, name="sd")
                t2list[g] = sd
            sd = t2list[g]
            sc = nc.gpsimd.local_scatter(
                out_ap=sd[:, t % 2, :], data_ap=wkTs[t // 4][:, t % 4, 0:100],
                idxs_ap=sidx[:],
                channels=128, num_elems=1536, num_idxs=100,
            )
            add_dep_helper(sc.ins, wkT_x[t // 4].ins,
                           reason="scatter reads xbar wkT")
            if sdst_reader[g % 3] is not None:
                add_dep_helper(sc.ins, sdst_reader[g % 3].ins,
                               reason="WAR: scatter overwrites xbar-read sdst")
            return sc

        def transpose_pair(g, sc0, sc1):
            T2 = sbw.tile([128, 24, 128], f16, tag="T2", bufs=3, name="T2")
            tx = nc.sync.dma_start_transpose(out=T2[:], in_=t2list[g][:])
            add_dep_helper(tx.ins, sc0.ins, reason="xbar reads scatter0")
            add_dep_helper(tx.ins, sc1.ins, reason="xbar reads scatter1")
            sdst_reader[g % 3] = tx
            return T2, tx

        def reassemble(t, T2, tx):
            for ch in range(2):
                po = ps.tile([128, 512], f32, tag="po", bufs=4, name="po")
                for dj in range(3):
                    pan = (t % 2) * 12 + dj * 4
                    mm_i = nc.tensor.matmul(
                        po[:], xt[:, t + dj, ch, :], T2[:, pan:pan + 4, :],
                        start=(dj == 0), stop=(dj == 2),
                    )
                    if ch == 0 and dj == 0:
                        add_dep_helper(mm_i.ins, tx.ins, reason="PE reads xbar T")
                if t % 4 == 0 and ch == 0:
                    osegs[(t // 4)] = sbw.tile([128, 8, 512], f16, tag="oseg",
                                               bufs=2, name="oseg")
                dst = osegs[t // 4][:, (t % 4) * 2 + ch, :]
                if ch == 0:
                    nc.scalar.activation(out=dst, in_=po[:],
                                         func=mybir.ActivationFunctionType.Copy,
                                         scale=1.0)
                else:
                    nc.vector.tensor_copy(dst, po[:])
            last = mm_i
            if t % 4 == 3:
                k = t // 4
                nc.scalar.dma_start(
                    out=d_out[:, k * 4096:(k + 1) * 4096],
                    in_=osegs[k][:],
                )
            return last

        # software-pipelined emission: pair g+1's scatters+transpose are
        # emitted before pair g's matmuls
        pend = {}
        last_pe = [None, None, None]   # T2 slot -> last PE reader
        sc0 = scatter(0)
        sc1 = scatter(1)
        pend[0] = transpose_pair(0, sc0, sc1)
        for g in range(8):
            # just-in-time wkT transposes slotted into the sync queue where
            # they are ready, so they never delay the T2 stream
            if g == 1:
                wkt_x(2)
            if g == 3:
                wkt_x(3)
            if g + 1 < 8:
                sc0 = scatter(2 * g + 2)
                sc1 = scatter(2 * g + 3)
                pend[g + 1] = transpose_pair(g + 1, sc0, sc1)
                if last_pe[(g + 1) % 3] is not None:
                    add_dep_helper(pend[g + 1][1].ins, last_pe[(g + 1) % 3].ins,
                                   reason="WAR: xbar overwrites PE-read T2")
            T2, tx = pend[g]
            reassemble(2 * g, T2, tx)
            if g == 0:
                # deferred tile-3 softmax tail: its e4 matmul waits on a
                # 3.3us DVE reciprocal; hide it behind pair 0's matmuls
                conv2_tail(3, recips[3])
            last_pe[g % 3] = reassemble(2 * g + 1, T2, tx)

    nc.compile()
    return nc


def _host_prep(x, W_comp, b_comp, W_enc, b_enc):
    """Build per-core input maps (layout/dtype prep only)."""
    idxs = _build_idxs()
    cst = np.zeros((128, C_TOT), np.float16)
    cst[0:128, C_WC0:C_WC0 + 64] = W_comp.T[0:128]
    cst[0:128, C_WC1:C_WC1 + 64] = W_comp.T[128:256]
    for j in range(3):
        cst[0:64, C_WEP + j * ENC:C_WEP + (j + 1) * ENC] = W_enc[:, :, 0, j].T
        cst[64:128, C_WEP + j * ENC:C_WEP + (j + 1) * ENC] = W_enc[:, :, 1, j].T
        cst[0:64, C_WES + j * ENC:C_WES + (j + 1) * ENC] = W_enc[:, :, 2, j].T
    for p in range(4):
        cst[C_ONES * 0 + p * 25:(p + 1) * 25, C_ONES + p] = 1.0
        cst[p, C_E4 + p * 25:C_E4 + (p + 1) * 25] = 1.0
    cstf = np.zeros((128, 2), np.float32)
    cstf[0:MID, 0] = b_comp
    cstf[0:ENC, 1] = b_enc

    xp = np.pad(x, ((0, 0), (0, 0), (2, 2), (0, 0)))   # (B, C, 68, 64)
    in_maps = []
    for core in range(8):
        b, half = core // 2, core % 2
        r0 = 32 * half
        xs = np.ascontiguousarray(
            xp[b, :, r0:r0 + NROW, :].reshape(C, NPX)).astype(np.float16)
        xflat = np.ascontiguousarray(xs.reshape(2, 128, NPX).transpose(1, 0, 2)
                                     ).reshape(128, 2 * NPX)
        xtc = np.ascontiguousarray(
            xs.reshape(2, 128, NJB, 128).transpose(3, 2, 0, 1)
        ).reshape(128, NJB * 2 * 128)
        in_maps.append(dict(x=xflat, xt=xtc, cst=cst, cstf=cstf, idx=idxs))
    return in_maps


def _gather(res):
    """Assemble full (B, C, 128, 128) fp32 output from per-core raw stores.

    Per core: out [128, 32, 512] f16 where free = (t, ch2, p, rt, w) with
    po free = (p, rt*64+w). Output pixel-shuffle: input row i = 2t+rt of the
    half; out rows 2i + (4w+p)//128, col (4w+p) % 128.
    """
    out = np.empty((B, C, 128, 128), np.float32)
    for core in range(8):
        b, half = core // 2, core % 2
        a = res.results[core]["out"].astype(np.float32)
        a = a.reshape(128, 16, 2, 4, 2, 64)      # (cp, t, c2, p, rt, w)
        a = a.transpose(1, 4, 2, 0, 5, 3)        # (t, rt, c2, cp, w, p)
        a = a.reshape(16, 2, 256, 2, 128)        # (t, rt, ch, rowbit, col)
        a = a.transpose(2, 0, 1, 3, 4).reshape(256, 64, 128)
        out[b, :, 64 * half:64 * (half + 1), :] = a
    return out


def kernel(x, W_comp, b_comp, W_enc, b_enc):
    x = np.asarray(x, np.float32)
    W_comp = np.asarray(W_comp, np.float32)
    b_comp = np.asarray(b_comp, np.float32)
    W_enc = np.asarray(W_enc, np.float32)
    b_enc = np.asarray(b_enc, np.float32)

    if "nc" not in _CACHE:
        _CACHE["nc"] = _build_nc()
    nc = _CACHE["nc"]

    in_maps = _host_prep(x, W_comp, b_comp, W_enc, b_enc)
    res = run_bass_kernel_spmd(nc, in_maps, core_ids=list(range(8)))
    return _gather(res)


if __name__ == "__main__":
    d = np.load("/tmp/carafe_ref.npz")
    expected = d["expected"]
    out = kernel(**{k: d[k] for k in ["x", "W_comp", "b_comp", "W_enc", "b_enc"]})
    err = np.abs(out - expected)
    scale = np.abs(expected).max()
    print(f"absmax err: {err.max():.4e}  rel: {err.max()/scale:.4e}")


# revision 11
# speedup vs baseline: 1.1877x; 1.0435x over previous
"""CARAFE++ content-aware upsampling kernel for Trainium2 (8 NeuronCores), v4.

Per-core pipeline (4 batches x 2 row-halves):
  1. conv1 as matmul (fp16) + relu -> featd: W-padded feat in partitions 0-63,
     one-row-shifted copy in partitions 64-127 (for conv2 row-pair packing)
  2. conv2 as 6 shifted matmuls per 8-row tile (3x K=128 row-pairs + 3x K=64),
     + bias + exp -> wk
  3. softmax denominators via block-ones matmul; reciprocal; broadcast back to
     100 rows via a 0/1-matrix matmul; wk_n = wk * recip (all per conv tile)
  4. XBAR dma-transpose wk_n -> wkT (pixel-major), 1 per conv tile
  5. per block-pair: two gpsimd local_scatters build band-matrix-transpose
     layouts; ONE XBAR dma-transpose yields 24 S panels; 12 accumulated
     fp16 matmuls reassemble; evict fp16 contiguous (no interleave; the host
     does the pixel-shuffle) + grouped stores
v4 changes vs v3: x load split in 4 chunks (early conv1 start), softmax chain
software-pipelined so the DVE reciprocal never blocks the tensor queue,
tile-3 chain tail deferred past the first reassembly pair, wkT transposes
interleaved into the T2 stream just-in-time, contiguous evictions with
grouped (4-block) stores, host-side output unshuffle.
All XBAR transposes serialize on the sync HWDGE ring (HW hazard otherwise);
loads/stores ride the scalar ring. XBAR writes/reads get manual deps (Tile
does not track InstDmaTransposeAnt).
"""
import sys

sys.path.insert(0, "/opt/trn_rl_repo")

import numpy as np
from contextlib import ExitStack

import concourse.bass as bass
import concourse.bacc as bacc
import concourse.tile as tile
from concourse.tile import add_dep_helper
from concourse import mybir
from concourse.bass_utils import run_bass_kernel_spmd

B, C, H, W = 4, 256, 64, 64
SCALE, K, COMP, G = 2, 5, 4, 1
MID = 64
ENC = 100          # K*K*SCALE*SCALE
NROW = 36          # x rows per core (32 + 2 halo each side)
NPX = NROW * W     # 2304
FROW = 34          # feat rows r0-1 .. r0+32
FPW = W + 2        # 66, feat row W-padded
NBLK = 16          # output row-pair blocks per core
NJB = 18           # x row-pair panels per core
XSPLIT = 1600      # x column split point per half (covers conv1 tiles 0-2)

f32 = mybir.dt.float32
f16 = mybir.dt.float16
i16 = mybir.dt.int16

# const_f16 blob column layout
C_WC0, C_WC1, C_WEP, C_WES, C_ONES, C_E4 = 0, 64, 128, 428, 728, 732
C_TOT = 832

_CACHE = {}


def _build_idxs():
    """Per-partition scatter indices. Partition = out-center px (rt, w); slot
    = (p, dy, dx) wk channel order; dest = (dj*4+p)*128 + rb*64 + wi so the
    XBAR panel-major transpose yields S panels grouped (dj, p)."""
    idxs = np.full((128, 100), -1, np.int16)
    for rt in range(2):
        for w in range(W):
            part = rt * W + w
            for p in range(4):
                for dy in range(-2, 3):
                    dj = (rt + dy + 2) // 2
                    rb = (rt + dy) % 2
                    for dx in range(-2, 3):
                        wi = w + dx
                        if 0 <= wi < W:
                            slot = p * 25 + (dy + 2) * 5 + (dx + 2)
                            idxs[part, slot] = (dj * 4 + p) * 128 + rb * 64 + wi
    return idxs


def _build_nc():
    nc = bacc.Bacc("TRN2", target_bir_lowering=False, debug=False, num_devices=8)

    # ---- DRAM I/O (per-core shapes)
    d_x = nc.dram_tensor("x", [128, 2 * NPX], f16, kind="ExternalInput")
    d_xt = nc.dram_tensor("xt", [128, NJB * 2 * 128], f16, kind="ExternalInput")
    d_cst = nc.dram_tensor("cst", [128, C_TOT], f16, kind="ExternalInput")
    d_cstf = nc.dram_tensor("cstf", [128, 2], f32, kind="ExternalInput")
    d_idx = nc.dram_tensor("idx", [128, 100], i16, kind="ExternalInput")
    d_out = nc.dram_tensor("out", [128, 2 * NBLK * 512], f16,
                           kind="ExternalOutput")

    with tile.TileContext(nc) as tc, ExitStack() as ctx:
        sb1 = ctx.enter_context(tc.tile_pool(name="sb1", bufs=1))
        sbw = ctx.enter_context(tc.tile_pool(name="sbw", bufs=2))
        ps = ctx.enter_context(tc.tile_pool(name="ps", bufs=1, space="PSUM"))

        # ---- loads. scalar ring: weights first, then x halves (conv1 tile 0
        # only gates on the "a" chunks); sync ring: xt (consumed late).
        cst = sb1.tile([128, C_TOT], f16, tag="cst")
        cstf = sb1.tile([128, 2], f32, tag="cstf")
        sidx = sb1.tile([128, 100], i16, tag="sidx")
        xh = []   # [h][chunk] tiles
        nc.scalar.dma_start(out=cst, in_=d_cst[:])
        for h in range(2):
            a = sb1.tile([128, XSPLIT], f16, tag=f"xh{h}a")
            b = sb1.tile([128, NPX - XSPLIT], f16, tag=f"xh{h}b")
            xh.append((a, b))
        for h in range(2):
            nc.scalar.dma_start(out=xh[h][0],
                                in_=d_x[:, h * NPX:h * NPX + XSPLIT])
        nc.scalar.dma_start(out=cstf, in_=d_cstf[:])
        nc.scalar.dma_start(out=sidx, in_=d_idx[:])
        for h in range(2):
            nc.scalar.dma_start(out=xh[h][1],
                                in_=d_x[:, h * NPX + XSPLIT:(h + 1) * NPX])
        # NOTE: load order already prioritizes cst + x "a" chunks
        xt = sb1.tile([128, NJB, 2, 128], f16, tag="xt")
        nc.sync.dma_start(
            out=xt, in_=d_xt[:].rearrange("q (j c m) -> q j c m", j=NJB, c=2))

        bc = cstf[0:MID, 0:1]
        be = cstf[0:ENC, 1:2]
        wc0 = cst[:, C_WC0:C_WC0 + 64]
        wc1 = cst[:, C_WC1:C_WC1 + 64]
        ones = cst[0:ENC, C_ONES:C_ONES + 4]
        e4 = cst[0:4, C_E4:C_E4 + ENC]

        # warm the Exp activation table during load shadow
        scratch = sb1.tile([ENC, 1], f32, tag="scratch")
        nc.scalar.activation(out=scratch[:], in_=be,
                             func=mybir.ActivationFunctionType.Exp,
                             bias=be, scale=1.0)

        featd = sb1.tile([128, FROW * FPW], f16, tag="featd")
        nc.vector.memset(featd, 0.0)
        wk = sb1.tile([ENC, 2048], f16, tag="wk")
        wkn = sb1.tile([112, 2048], f16, tag="wkn")
        nc.vector.memset(wkn[96:112, :], 0.0)
        wkTs = [None] * 4  # per conv tile [128, 4, 112]
        wkT_x = [None] * 16  # per BLOCK: transpose instr covering that block
        mul_is = [None] * 4

        def xsrc(h, n0, n):
            """x slice [n0, n0+n) of half h from the right chunk tile."""
            if n0 + n <= XSPLIT:
                return xh[h][0][:, n0:n0 + n]
            return xh[h][1][:, n0 - XSPLIT:n0 - XSPLIT + n]

        # ---- conv1 tile: 1x1 conv (256->64) + relu -> featd (both halves)
        def conv1_tile(nt):
            n0 = W + nt * 512
            n = min(512, 2240 - n0)
            pf = ps.tile([MID, 512], f32, tag="pf", bufs=2, name="pf")
            nc.tensor.matmul(pf[:, :n], wc0, xsrc(0, n0, n),
                             start=True, stop=False)
            nc.tensor.matmul(pf[:, :n], wc1, xsrc(1, n0, n),
                             start=False, stop=True)
            fp0 = n0 // W - 1
            nr = n // W
            src = pf[:, :n].rearrange("m (r w) -> m r w", w=W)
            fd1h = featd[0:64]
            dst1 = bass.AP(
                tensor=fd1h.tensor, offset=fd1h.offset + fp0 * FPW + 1,
                ap=[fd1h.ap[0], [FPW, nr], [1, W]],
            )
            nc.scalar.activation(out=dst1, in_=src,
                                 func=mybir.ActivationFunctionType.Relu,
                                 bias=bc, scale=1.0)
            fd2 = featd[64:128]
            if fp0 == 0:
                src2 = bass.AP(tensor=pf.tensor, offset=pf.offset + W,
                               ap=[pf.ap[0], [W, nr - 1], [1, W]])
                dst2 = bass.AP(tensor=fd2.tensor, offset=fd2.offset + 1,
                               ap=[fd2.ap[0], [FPW, nr - 1], [1, W]])
            else:
                src2 = bass.AP(tensor=pf.tensor, offset=pf.offset,
                               ap=[pf.ap[0], [W, nr], [1, W]])
                dst2 = bass.AP(tensor=fd2.tensor,
                               offset=fd2.offset + (fp0 - 1) * FPW + 1,
                               ap=[fd2.ap[0], [FPW, nr], [1, W]])
            nc.scalar.activation(out=dst2, in_=src2,
                                 func=mybir.ActivationFunctionType.Relu,
                                 bias=bc, scale=1.0)

        # ---- conv2 pieces, split so the softmax chain pipelines across
        # tiles without ever stalling the tensor queue.
        def conv2_mm(nt):
            h0 = nt * 8
            sl = slice(nt * 512, (nt + 1) * 512)
            pw = ps.tile([ENC, 512], f32, tag="pw", bufs=1, name="pw")
            for j in range(3):
                rhs = bass.AP(
                    tensor=featd.tensor, offset=featd.offset + h0 * FPW + j,
                    ap=[featd.ap[0], [FPW, 8], [1, W]],
                )
                nc.tensor.matmul(pw[:], cst[:, C_WEP + j * ENC:C_WEP + (j + 1) * ENC],
                                 rhs, start=(j == 0), stop=False)
            fd1 = featd[0:64]
            for j in range(3):
                rhs = bass.AP(
                    tensor=fd1.tensor,
                    offset=fd1.offset + (h0 + 2) * FPW + j,
                    ap=[fd1.ap[0], [FPW, 8], [1, W]],
                )
                nc.tensor.matmul(pw[:], cst[0:64, C_WES + j * ENC:C_WES + (j + 1) * ENC],
                                 rhs, start=False, stop=(j == 2))
            return pw

        def conv2_exp(nt, pw):
            sl = slice(nt * 512, (nt + 1) * 512)
            nc.scalar.activation(out=wk[:, sl], in_=pw[:],
                                 func=mybir.ActivationFunctionType.Exp,
                                 bias=be, scale=1.0)

        def conv2_ones(nt):
            sl = slice(nt * 512, (nt + 1) * 512)
            paux = ps.tile([ENC, 512], f32, tag="paux", bufs=1, name="paux")
            nc.tensor.matmul(paux[0:4, :], ones, wk[:, sl], start=True, stop=True)
            recip4 = sbw.tile([4, 512], f16, tag="recip4", bufs=2, name="recip4")
            return recip4, paux

        def conv2_recip(recip4, paux, c0, c1):
            with nc.allow_low_precision(reason="softmax recip fine in fp16"):
                nc.vector.reciprocal(recip4[:, c0:c1], paux[0:4, c0:c1])

        def conv2_tail(nt, recip4, c0=0, c1=512, paux2=None):
            """e4 broadcast matmul + normalize for tile nt, cols [c0,c1)."""
            sl = slice(nt * 512 + c0, nt * 512 + c1)
            if paux2 is None:
                paux2 = ps.tile([ENC, 512], f32, tag="paux", bufs=1, name="paux2")
            nc.tensor.matmul(paux2[:, c0:c1], e4, recip4[:, c0:c1],
                             start=True, stop=True)
            mul_i = nc.vector.tensor_mul(wkn[0:ENC, sl], wk[:, sl],
                                         paux2[:, c0:c1])
            mul_is[nt] = mul_i
            return mul_i, paux2

        def wkt_x(nt, half=None, mul_i=None):
            """XBAR transpose wkn -> wkT for conv tile nt (or one px-half)."""
            if half is None or half == 0:
                wkT = sbw.tile([128, 4, 112], f16, tag="wkT", bufs=4, name="wkT")
                wkTs[nt] = wkT
            wkT = wkTs[nt]
            if half is None:
                sl = slice(nt * 512, (nt + 1) * 512)
                dst, blks = wkT[:], range(nt * 4, nt * 4 + 4)
            else:
                sl = slice(nt * 512 + half * 256, nt * 512 + half * 256 + 256)
                dst = wkT[:, half * 2:half * 2 + 2, :]
                blks = range(nt * 4 + half * 2, nt * 4 + half * 2 + 2)
            wx = nc.sync.dma_start_transpose(out=dst, in_=wkn[:, sl])
            # Tile does not dep-track XBAR transposes: manual edge
            add_dep_helper(wx.ins, (mul_i or mul_is[nt]).ins,
                           reason="xbar wkT reads wkn")
            for t in blks:
                wkT_x[t] = wx
            return wx

        # ---- conv phase emission: tensor queue stays dense; each tile's
        # reciprocal latency is hidden under the next tile's conv matmuls.
        # Tile 0's softmax chain is split in px-halves (half-size reciprocal,
        # two half wkT transposes) to start the scatter/T2 stream earlier.
        conv1_tile(0)
        conv1_tile(1)
        recips = [None] * 4
        pw0 = conv2_mm(0)
        conv2_exp(0, pw0)
        conv1_tile(2)
        recips[0] = conv2_ones(0)
        conv2_recip(*recips[0], 0, 256)
        m0a, px2_0 = conv2_tail(0, recips[0][0], 0, 256)
        w0a = wkt_x(0, half=0, mul_i=m0a)
        conv2_recip(*recips[0], 256, 512)
        m0b, _ = conv2_tail(0, recips[0][0], 256, 512, paux2=px2_0)
        w0b = wkt_x(0, half=1, mul_i=m0b)
        pw1 = conv2_mm(1)
        conv2_exp(1, pw1)
        conv1_tile(3)
        recips[1] = conv2_ones(1)
        conv2_recip(*recips[1], 0, 512)
        pw2 = conv2_mm(2)
        conv2_tail(1, recips[1][0])
        w1 = wkt_x(1)
        conv2_exp(2, pw2)
        conv1_tile(4)
        recips[2] = conv2_ones(2)
        conv2_recip(*recips[2], 0, 512)
        pw3 = conv2_mm(3)
        conv2_tail(2, recips[2][0])
        conv2_exp(3, pw3)
        recips[3] = conv2_ones(3)
        conv2_recip(*recips[3], 0, 512)
        # tile 3's tail (e4 matmul gated on the DVE reciprocal) is deferred
        # into the reassembly loop so it never stalls the PE.

        # ---- reassembly: per block scatter; per pair one XBAR transpose,
        # 12 matmuls, contiguous fp16 evictions, grouped stores.
        sdst_reader = [None] * 4
        t2list = [None] * 8
        osegs = [None] * 4

        def scatter(t):
            g = t // 2
            if t % 2 == 0:
                sd = sbw.tile([128, 2, 1536], f16, tag="sdst", bufs=4, name="sd")
                t2list[g] = sd
            sd = t2list[g]
            sc = nc.gpsimd.local_scatter(
                out_ap=sd[:, t % 2, :], data_ap=wkTs[t // 4][:, t % 4, 0:100],
                idxs_ap=sidx[:],
                channels=128, num_elems=1536, num_idxs=100,
            )
            add_dep_helper(sc.ins, wkT_x[t].ins,
                           reason="scatter reads xbar wkT")
            if sdst_reader[g % 4] is not None:
                add_dep_helper(sc.ins, sdst_reader[g % 4].ins,
                               reason="WAR: scatter overwrites xbar-read sdst")
            return sc

        def transpose_pair(g, sc0, sc1):
            T2 = sbw.tile([128, 24, 128], f16, tag="T2", bufs=4, name="T2")
            tx = nc.sync.dma_start_transpose(out=T2[:], in_=t2list[g][:])
            add_dep_helper(tx.ins, sc0.ins, reason="xbar reads scatter0")
            add_dep_helper(tx.ins, sc1.ins, reason="xbar reads scatter1")
            sdst_reader[g % 4] = tx
            return T2, tx

        def reassemble(t, T2, tx):
            for ch in range(2):
                po = ps.tile([128, 512], f32, tag="po", bufs=4, name="po")
                for dj in range(3):
                    pan = (t % 2) * 12 + dj * 4
                    mm_i = nc.tensor.matmul(
                        po[:], xt[:, t + dj, ch, :], T2[:, pan:pan + 4, :],
                        start=(dj == 0), stop=(dj == 2),
                    )
                    if ch == 0 and dj == 0:
                        add_dep_helper(mm_i.ins, tx.ins, reason="PE reads xbar T")
                if t % 4 == 0 and ch == 0:
                    osegs[(t // 4)] = sbw.tile([128, 8, 512], f16, tag="oseg",
                                               bufs=2, name="oseg")
                dst = osegs[t // 4][:, (t % 4) * 2 + ch, :]
                if ch == 0:
                    nc.scalar.activation(out=dst, in_=po[:],
                                         func=mybir.ActivationFunctionType.Copy,
                                         scale=1.0)
                else:
                    nc.vector.tensor_copy(dst, po[:])
            last = mm_i
            if t % 4 == 3:
                k = t // 4
                nc.scalar.dma_start(
                    out=d_out[:, k * 4096:(k + 1) * 4096],
                    in_=osegs[k][:],
                )
            return last

        # software-pipelined emission: pair g+1's scatters+transpose are
        # emitted before pair g's matmuls
        pend = {}
        last_pe = [None] * 4   # T2 slot -> last PE reader
        sc0 = scatter(0)
        sc1 = scatter(1)
        pend[0] = transpose_pair(0, sc0, sc1)
        # pin late wkT transposes behind T2 transposes on the sync ring so
        # the scheduler can never stall the T2 stream on a wkT's inputs
        add_dep_helper(w1.ins, pend[0][1].ins, reason="order wkT1 after T2x0")
        for g in range(8):
            if g == 1:
                w2 = wkt_x(2)
                add_dep_helper(w2.ins, pend[1][1].ins,
                               reason="order wkT2 after T2x1")
            if g == 3:
                w3 = wkt_x(3)
                add_dep_helper(w3.ins, pend[3][1].ins,
                               reason="order wkT3 after T2x3")
            if g + 1 < 8:
                sc0 = scatter(2 * g + 2)
                sc1 = scatter(2 * g + 3)
                pend[g + 1] = transpose_pair(g + 1, sc0, sc1)
                if last_pe[(g + 1) % 4] is not None:
                    add_dep_helper(pend[g + 1][1].ins, last_pe[(g + 1) % 4].ins,
                                   reason="WAR: xbar overwrites PE-read T2")
            T2, tx = pend[g]
            reassemble(2 * g, T2, tx)
            if g == 0:
                # deferred tile-3 softmax tail: its e4 matmul waits on the
                # DVE reciprocal; hide it behind pair 0's matmuls
                conv2_tail(3, recips[3][0])
            last_pe[g % 4] = reassemble(2 * g + 1, T2, tx)

    nc.compile()
    return nc


def _host_prep(x, W_comp, b_comp, W_enc, b_enc):
    """Build per-core input maps (layout/dtype prep only)."""
    idxs = _build_idxs()
    cst = np.zeros((128, C_TOT), np.float16)
    cst[0:128, C_WC0:C_WC0 + 64] = W_comp.T[0:128]
    cst[0:128, C_WC1:C_WC1 + 64] = W_comp.T[128:256]
    for j in range(3):
        cst[0:64, C_WEP + j * ENC:C_WEP + (j + 1) * ENC] = W_enc[:, :, 0, j].T
        cst[64:128, C_WEP + j * ENC:C_WEP + (j + 1) * ENC] = W_enc[:, :, 1, j].T
        cst[0:64, C_WES + j * ENC:C_WES + (j + 1) * ENC] = W_enc[:, :, 2, j].T
    for p in range(4):
        cst[C_ONES * 0 + p * 25:(p + 1) * 25, C_ONES + p] = 1.0
        cst[p, C_E4 + p * 25:C_E4 + (p + 1) * 25] = 1.0
    cstf = np.zeros((128, 2), np.float32)
    cstf[0:MID, 0] = b_comp
    cstf[0:ENC, 1] = b_enc

    xp = np.pad(x, ((0, 0), (0, 0), (2, 2), (0, 0)))   # (B, C, 68, 64)
    in_maps = []
    for core in range(8):
        b, half = core // 2, core % 2
        r0 = 32 * half
        xs = np.ascontiguousarray(
            xp[b, :, r0:r0 + NROW, :].reshape(C, NPX)).astype(np.float16)
        xflat = np.ascontiguousarray(xs.reshape(2, 128, NPX).transpose(1, 0, 2)
                                     ).reshape(128, 2 * NPX)
        xtc = np.ascontiguousarray(
            xs.reshape(2, 128, NJB, 128).transpose(3, 2, 0, 1)
        ).reshape(128, NJB * 2 * 128)
        in_maps.append(dict(x=xflat, xt=xtc, cst=cst, cstf=cstf, idx=idxs))
    return in_maps


def _gather(res):
    """Assemble full (B, C, 128, 128) fp32 output from per-core raw stores.

    Per core: out [128, 32, 512] f16 where free = (t, ch2, p, rt, w) with
    po free = (p, rt*64+w). Output pixel-shuffle: input row i = 2t+rt of the
    half; out rows 2i + (4w+p)//128, col (4w+p) % 128.
    """
    out = np.empty((B, C, 128, 128), np.float32)
    for core in range(8):
        b, half = core // 2, core % 2
        a = res.results[core]["out"].astype(np.float32)
        a = a.reshape(128, 16, 2, 4, 2, 64)      # (cp, t, c2, p, rt, w)
        a = a.transpose(1, 4, 2, 0, 5, 3)        # (t, rt, c2, cp, w, p)
        a = a.reshape(16, 2, 256, 2, 128)        # (t, rt, ch, rowbit, col)
        a = a.transpose(2, 0, 1, 3, 4).reshape(256, 64, 128)
        out[b, :, 64 * half:64 * (half + 1), :] = a
    return out


def kernel(x, W_comp, b_comp, W_enc, b_enc):
    x = np.asarray(x, np.float32)
    W_comp = np.asarray(W_comp, np.float32)
    b_comp = np.asarray(b_comp, np.float32)
    W_enc = np.asarray(W_enc, np.float32)
    b_enc = np.asarray(b_enc, np.float32)

    if "nc" not in _CACHE:
        _CACHE["nc"] = _build_nc()
    nc = _CACHE["nc"]

    in_maps = _host_prep(x, W_comp, b_comp, W_enc, b_enc)
    res = run_bass_kernel_spmd(nc, in_maps, core_ids=list(range(8)))
    return _gather(res)


if __name__ == "__main__":
    d = np.load("/tmp/carafe_ref.npz")
    expected = d["expected"]
    out = kernel(**{k: d[k] for k in ["x", "W_comp", "b_comp", "W_enc", "b_enc"]})
    err = np.abs(out - expected)
    scale = np.abs(expected).max()
    print(f"absmax err: {err.max():.4e}  rel: {err.max()/scale:.4e}")
